# revision 2
# baseline (speedup 1.0000x reference)
"""Trainium2 Bass/Tile kernel for the AttentionModule problem.

Computation (per the reference):
    h_proj  = hidden @ Wa[:, :D].T + ba                       [B, 2E]
    e_proj  = einsum('tbe,fe->tbf', enc, Wa[:, D:])           [T, B, 2E]
    act     = tanh(h_proj + e_proj)
    scores  = einsum('tbf,f->bt', act, w2[0]) (+ b2, dropped — softmax invariant)
    weights = softmax(scores, axis=t)
    applied = einsum('bt,tbe->be', weights, enc)              [B, E]
    out     = tanh(cat(decoder_out, applied) @ Wc.T + bc)     [B, D]

Strategy: data-parallel over batch B=64 across 8 NeuronCores (8 rows each).
All matmul operands are host-pre-transposed so the contraction dim (e/d/k)
lands on SBUF partitions; inputs are cast to bf16 on host (fp32 PSUM accum).

Per-core device layout (f = 2E on PSUM partitions for the big matmul):
    pre[f_tile=128, t=512]  = sum_e WaET[e, f].T @ encT[e, (b,t)]  (8 K-tiles)
    act = tanh(pre + h_projT[f, b])        (ACT, bias = per-partition scalar)
    scores[1, t] += w2[f_tile].T @ act     (PE, M=1, accumulated over 16 f-tiles)
    softmax on one partition, weights broadcast to 128 partitions via DMA
    appliedT[e_tile, b] = reduce_t(encT * wrep)   (DVE tensor_tensor_reduce)
    out[b, :] = tanh(cat(decT, appliedT).T @ WcT + bc)  (PE + ACT)
"""

import numpy as np
import ml_dtypes
from contextlib import ExitStack

import concourse.bass as bass
import concourse.tile as tile
from concourse import bacc, mybir
from concourse.bass_utils import run_bass_kernel_spmd
from concourse.masks import make_identity

B, T, E, D = 64, 512, 1024, 1024
NCORES = 8
BL = B // NCORES          # 8 batch rows per core
F = 2 * E                 # 2048
KE = E // 128             # 8 contraction tiles for e/d
KC = (D + E) // 128       # 16 contraction tiles for the combine matmul
FJ = F // 128             # 16 f-tiles
BF16 = mybir.dt.bfloat16
F32 = mybir.dt.float32
AF = mybir.ActivationFunctionType
ALU = mybir.AluOpType

_nc_cache = None


def _body(tc, ins, wscr, out_d, app_d):
    nc = tc.nc
    with ExitStack() as ctx:
        const = ctx.enter_context(tc.tile_pool(name="const", bufs=1))
        act_pool = ctx.enter_context(tc.tile_pool(name="actp", bufs=3))
        wrep_pool = ctx.enter_context(tc.tile_pool(name="wrepp", bufs=2))
        scr_pool = ctx.enter_context(tc.tile_pool(name="scrp", bufs=2))
        sm_pool = ctx.enter_context(tc.tile_pool(name="smp", bufs=2))
        pe_psum = ctx.enter_context(tc.tile_pool(name="pep", bufs=3, space="PSUM"))
        ps_psum = ctx.enter_context(tc.tile_pool(name="psp", bufs=2, space="PSUM"))
        misc_psum = ctx.enter_context(tc.tile_pool(name="mip", bufs=2, space="PSUM"))

        # ---- constants / weights into SBUF ----
        ones = const.tile([1, BL], BF16, name="ones", tag="ones")
        nc.vector.memset(ones, 1.0)
        ident = const.tile([128, 128], F32, name="ident", tag="ident")
        make_identity(nc, ident)

        hT_sb = []
        waDT_sb = []
        for k in range(KE):
            t_h = const.tile([128, BL], BF16, name=f"hT{k}", tag=f"hT{k}")
            nc.sync.dma_start(out=t_h, in_=ins["hT"][k * 128:(k + 1) * 128, :])
            hT_sb.append(t_h)
            t_w = const.tile([128, F], BF16, name=f"waDT{k}", tag=f"waDT{k}")
            nc.sync.dma_start(out=t_w, in_=ins["WaDT"][k * 128:(k + 1) * 128, :])
            waDT_sb.append(t_w)
        ba_sb = const.tile([1, F], BF16, name="ba_sb", tag="ba")
        nc.sync.dma_start(out=ba_sb, in_=ins["baR"])
        waET_sb = []
        for k in range(KE):
            t_w = const.tile([128, F], BF16, name=f"waET{k}", tag=f"waET{k}")
            nc.sync.dma_start(out=t_w, in_=ins["WaET"][k * 128:(k + 1) * 128, :])
            waET_sb.append(t_w)
        w2_sb = const.tile([128, FJ], BF16, name="w2_sb", tag="w2")
        nc.sync.dma_start(out=w2_sb, in_=ins["w2T"])

        # encoder states, transposed: [e, b*T + t]; per-(k, b) tiles so the
        # first batch column can start computing before the full load lands
        enc_sb = [[None] * BL for _ in range(KE)]
        for b in range(BL):
            for k in range(KE):
                t_e = const.tile([128, T], BF16, name=f"enc{k}_{b}", tag=f"enc{k}_{b}")
                nc.sync.dma_start(
                    out=t_e,
                    in_=ins["encT"][k * 128:(k + 1) * 128, b * T:(b + 1) * T],
                )
                enc_sb[k][b] = t_e

        decT_sb = []
        for k in range(KE):
            t_d = const.tile([128, BL], BF16, name=f"decT{k}", tag=f"decT{k}")
            nc.sync.dma_start(out=t_d, in_=ins["decT"][k * 128:(k + 1) * 128, :])
            decT_sb.append(t_d)
        wcT_sb = []
        for k in range(KC):
            t_w = const.tile([128, D], BF16, name=f"wcT{k}", tag=f"wcT{k}")
            nc.sync.dma_start(out=t_w, in_=ins["WcT"][k * 128:(k + 1) * 128, :])
            wcT_sb.append(t_w)
        bc_sb = const.tile([1, D], BF16, name="bc_sb", tag="bc")
        nc.sync.dma_start(out=bc_sb, in_=ins["bcR"])

        # ---- h_proj = hidden @ WaD.T + ba  → [BL, F] then transpose ----
        h_proj = const.tile([BL, F], F32, name="h_proj", tag="h_proj")
        for c in range(F // 512):
            ph = misc_psum.tile([BL, 512], F32, name=f"ph{c}", tag="misc")
            for k in range(KE):
                nc.tensor.matmul(
                    ph, hT_sb[k], waDT_sb[k][:, c * 512:(c + 1) * 512],
                    start=(k == 0), stop=False,
                )
            nc.tensor.matmul(
                ph, ones, ba_sb[:, c * 512:(c + 1) * 512], start=False, stop=True,
            )
            nc.scalar.copy(h_proj[:, c * 512:(c + 1) * 512], ph)

        # h_projT[f, j, b] per-partition bias layout: [128, FJ, BL]
        h_projT = const.tile([128, FJ, BL], F32, name="h_projT", tag="h_projT")
        for j in range(FJ):
            pt = misc_psum.tile([128, BL], F32, name=f"pt{j}", tag="misc")
            nc.tensor.transpose(pt, h_proj[:, j * 128:(j + 1) * 128], ident[0:BL, 0:BL])
            nc.scalar.copy(h_projT[:, j, :], pt)

        # appliedT[e_tile][128, BL] accumulators (fp32)
        appT = []
        for k in range(KE):
            t_a = const.tile([128, BL], F32, name=f"appT{k}", tag=f"appT{k}")
            appT.append(t_a)

        # ---- main loop: per batch row, per f-tile ----
        for b in range(BL):
            ps = ps_psum.tile([1, T], F32, name=f"ps{b}", tag="ps")
            score_mm = []  # deferred score matmuls, emitted one j behind
            for j in range(FJ):
                pe = pe_psum.tile([128, T], F32, name=f"pe{b}_{j}", tag="pe")
                for k in range(KE):
                    nc.tensor.matmul(
                        pe,
                        waET_sb[k][:, j * 128:(j + 1) * 128],
                        enc_sb[k][b],
                        start=(k == 0), stop=(k == KE - 1),
                    )
                a_t = act_pool.tile([128, T], BF16, name=f"act{b}_{j}", tag="act")
                nc.scalar.activation(
                    a_t, pe, AF.Tanh, bias=h_projT[:, j, b:b + 1],
                )
                score_mm.append(a_t)
                # emit the score matmul for the *previous* j so PE doesn't
                # stall waiting on this j's tanh
                if j > 0:
                    nc.tensor.matmul(
                        ps, w2_sb[:, j - 1:j], score_mm[j - 1],
                        start=(j - 1 == 0), stop=False,
                    )
            nc.tensor.matmul(
                ps, w2_sb[:, FJ - 1:FJ], score_mm[FJ - 1], start=False, stop=True,
            )

            # softmax over t on a single partition
            negmax = sm_pool.tile([1, 1], F32, name=f"negmax{b}", tag="negmax")
            nc.vector.reduce_max(negmax, ps, axis=mybir.AxisListType.X, negate=True)
            wexp = sm_pool.tile([1, T], F32, name=f"wexp{b}", tag="wexp")
            sume = sm_pool.tile([1, 1], F32, name=f"sume{b}", tag="sume")
            nc.scalar.activation(wexp, ps, AF.Exp, bias=negmax, accum_out=sume)
            rsum = sm_pool.tile([1, 1], F32, name=f"rsum{b}", tag="rsum")
            nc.vector.reciprocal(rsum, sume)
            wnorm = sm_pool.tile([1, T], BF16, name=f"wnorm{b}", tag="wnorm")
            nc.vector.tensor_scalar_mul(wnorm, wexp, rsum)

            # broadcast weights to 128 partitions via DRAM round-trip
            nc.sync.dma_start(out=wscr[b:b + 1, :], in_=wnorm)
            wrep = wrep_pool.tile([128, T], BF16, name=f"wrep{b}", tag="wrep")
            row = wscr[b:b + 1, :]
            bsrc = bass.AP(tensor=row.tensor, offset=row.offset,
                           ap=[[0, 128]] + [list(p) for p in row.ap[1:]])
            nc.sync.dma_start(out=wrep, in_=bsrc)

            # appliedT[:, b] = sum_t enc * w
            for k in range(KE):
                scr = scr_pool.tile([128, T], BF16, name=f"scr{b}_{k}", tag="scr")
                nc.vector.scalar_tensor_tensor(
                    out=scr, in0=enc_sb[k][b], scalar=1.0, in1=wrep,
                    op0=ALU.mult, op1=ALU.mult,
                    accum_out=appT[k][:, b:b + 1],
                )

        # ---- epilogue: applied output + combine matmul ----
        applied_sb = const.tile([BL, E], F32, name="applied_sb", tag="applied_sb")
        appT_bf = []
        for k in range(KE):
            t_c = const.tile([128, BL], BF16, name=f"appBf{k}", tag=f"appBf{k}")
            nc.vector.tensor_copy(t_c, appT[k])
            appT_bf.append(t_c)
            pa = misc_psum.tile([BL, 128], F32, name=f"pa{k}", tag="misc")
            nc.tensor.transpose(pa, appT[k], ident)
            nc.scalar.copy(applied_sb[:, k * 128:(k + 1) * 128], pa)
        nc.sync.dma_start(out=app_d, in_=applied_sb)

        out_sb = const.tile([BL, D], F32, name="out_sb", tag="out_sb")
        for h in range(D // 512):
            pc = misc_psum.tile([BL, 512], F32, name=f"pc{h}", tag="misc")
            for k in range(KC):
                lhs = decT_sb[k] if k < KE else appT_bf[k - KE]
                nc.tensor.matmul(
                    pc, lhs, wcT_sb[k][:, h * 512:(h + 1) * 512],
                    start=(k == 0), stop=False,
                )
            nc.tensor.matmul(
                pc, ones, bc_sb[:, h * 512:(h + 1) * 512], start=False, stop=True,
            )
            nc.scalar.activation(out_sb[:, h * 512:(h + 1) * 512], pc, AF.Tanh)
        nc.sync.dma_start(out=out_d, in_=out_sb)


def build_nc():
    nc = bacc.Bacc("TRN2", target_bir_lowering=False, debug=False)
    ins = {}

    def din(name, shape, dt=BF16):
        ins[name] = nc.dram_tensor(name, shape, dt, kind="ExternalInput").ap()

    din("encT", [E, BL * T])
    din("hT", [D, BL])
    din("decT", [D, BL])
    din("WaDT", [D, F])
    din("WaET", [E, F])
    din("WcT", [D + E, D])
    din("w2T", [128, FJ])
    din("baR", [1, F])
    din("bcR", [1, D])
    wscr = nc.dram_tensor("wscr", [BL, T], BF16, kind="Internal").ap()
    out_d = nc.dram_tensor("out", [BL, D], F32, kind="ExternalOutput").ap()
    app_d = nc.dram_tensor("applied", [BL, E], F32, kind="ExternalOutput").ap()
    with tile.TileContext(nc) as tc:
        _body(tc, ins, wscr, out_d, app_d)
    nc.compile()
    return nc


def _prep_inputs(hidden, decoder_out, encoder_states, Wa, ba, w2, Wc, bc):
    bf = ml_dtypes.bfloat16
    f32 = np.float32

    def to_bf(a):
        return np.ascontiguousarray(np.asarray(a, f32)).astype(bf)

    shared = {
        "WaDT": np.ascontiguousarray(np.asarray(Wa[:, :D], f32).T).astype(bf),
        "WaET": np.ascontiguousarray(np.asarray(Wa[:, D:], f32).T).astype(bf),
        "WcT": np.ascontiguousarray(np.asarray(Wc, f32).T).astype(bf),
        "w2T": np.ascontiguousarray(
            np.asarray(w2[0], f32).reshape(FJ, 128).T).astype(bf),
        "baR": to_bf(np.asarray(ba, f32).reshape(1, F)),
        "bcR": to_bf(np.asarray(bc, f32).reshape(1, D)),
    }
    enc_bf = np.asarray(encoder_states, f32).astype(bf)  # [T, B, E]
    in_maps = []
    for c in range(NCORES):
        sl = slice(c * BL, (c + 1) * BL)
        encT = np.ascontiguousarray(
            enc_bf[:, sl, :].transpose(2, 1, 0)).reshape(E, BL * T)
        m = dict(shared)
        m["encT"] = encT
        m["hT"] = np.ascontiguousarray(np.asarray(hidden[sl], f32).T).astype(bf)
        m["decT"] = np.ascontiguousarray(np.asarray(decoder_out[sl], f32).T).astype(bf)
        in_maps.append(m)
    return in_maps


def kernel(hidden, decoder_out, encoder_states, Wa, ba, w2, b2, Wc, bc):
    global _nc_cache
    if _nc_cache is None:
        _nc_cache = build_nc()
    in_maps = _prep_inputs(hidden, decoder_out, encoder_states, Wa, ba, w2, Wc, bc)
    res = run_bass_kernel_spmd(_nc_cache, in_maps, core_ids=list(range(NCORES)))
    out = np.concatenate([res.results[c]["out"] for c in range(NCORES)], axis=0)
    applied = np.concatenate(
        [res.results[c]["applied"] for c in range(NCORES)], axis=0)
    return out.astype(np.float32), applied.astype(np.float32)


# revision 3
# speedup vs baseline: 121.1362x; 121.1362x over previous
"""Trainium2 Bass/Tile kernel for the AttentionModule problem.

Computation (per the reference):
    h_proj  = hidden @ Wa[:, :D].T + ba                       [B, 2E]
    e_proj  = einsum('tbe,fe->tbf', enc, Wa[:, D:])           [T, B, 2E]
    act     = tanh(h_proj + e_proj)
    scores  = einsum('tbf,f->bt', act, w2[0]) (+ b2, dropped — softmax invariant)
    weights = softmax(scores, axis=t)
    applied = einsum('bt,tbe->be', weights, enc)              [B, E]
    out     = tanh(cat(decoder_out, applied) @ Wc.T + bc)     [B, D]

Strategy: data-parallel over batch B=64 across 8 NeuronCores (8 rows each).
All matmul operands are host-pre-transposed so the contraction dim (e/d/k)
lands on SBUF partitions; inputs are cast to bf16 on host (fp32 PSUM accum).

Per-core device layout (f = 2E on PSUM partitions for the big matmul):
    pre[f_tile=128, t=512]  = sum_e WaET[e, f].T @ encT[e, (b,t)]  (8 K-tiles)
    act = tanh(pre + h_projT[f, b])        (ACT, bias = per-partition scalar)
    scores[1, t] += w2[f_tile].T @ act     (PE, M=1, accumulated over 16 f-tiles)
    softmax on one partition, weights broadcast to 128 partitions via DMA
    appliedT[e_tile, b] = reduce_t(encT * wrep)   (DVE tensor_tensor_reduce)
    out[b, :] = tanh(cat(decT, appliedT).T @ WcT + bc)  (PE + ACT)
"""

import numpy as np
import ml_dtypes
from contextlib import ExitStack

import concourse.bass as bass
import concourse.tile as tile
from concourse import bacc, mybir
from concourse.bass_utils import run_bass_kernel_spmd
from concourse.masks import make_identity

B, T, E, D = 64, 512, 1024, 1024
NCORES = 8
BL = B // NCORES          # 8 batch rows per core
F = 2 * E                 # 2048
KE = E // 128             # 8 contraction tiles for e/d
KC = (D + E) // 128       # 16 contraction tiles for the combine matmul
FJ = F // 128             # 16 f-tiles
BF16 = mybir.dt.bfloat16
F32 = mybir.dt.float32
AF = mybir.ActivationFunctionType
ALU = mybir.AluOpType

_nc_cache = None


def _body(tc, ins, wscr, out_d, app_d):
    nc = tc.nc
    with ExitStack() as ctx:
        const = ctx.enter_context(tc.tile_pool(name="const", bufs=1))
        act_pool = ctx.enter_context(tc.tile_pool(name="actp", bufs=3))
        wrep_pool = ctx.enter_context(tc.tile_pool(name="wrepp", bufs=2))
        scr_pool = ctx.enter_context(tc.tile_pool(name="scrp", bufs=2))
        sm_pool = ctx.enter_context(tc.tile_pool(name="smp", bufs=2))
        pe_psum = ctx.enter_context(tc.tile_pool(name="pep", bufs=3, space="PSUM"))
        ps_psum = ctx.enter_context(tc.tile_pool(name="psp", bufs=2, space="PSUM"))
        misc_psum = ctx.enter_context(tc.tile_pool(name="mip", bufs=2, space="PSUM"))

        # ---- constants / weights into SBUF ----
        ones = const.tile([1, BL], BF16, name="ones", tag="ones")
        nc.vector.memset(ones, 1.0)
        ident = const.tile([128, 128], F32, name="ident", tag="ident")
        make_identity(nc, ident)

        hT_sb = []
        waDT_sb = []
        for k in range(KE):
            t_h = const.tile([128, BL], BF16, name=f"hT{k}", tag=f"hT{k}")
            nc.sync.dma_start(out=t_h, in_=ins["hT"][k * 128:(k + 1) * 128, :])
            hT_sb.append(t_h)
            t_w = const.tile([128, F], BF16, name=f"waDT{k}", tag=f"waDT{k}")
            nc.sync.dma_start(out=t_w, in_=ins["WaDT"][k * 128:(k + 1) * 128, :])
            waDT_sb.append(t_w)
        ba_sb = const.tile([1, F], BF16, name="ba_sb", tag="ba")
        nc.sync.dma_start(out=ba_sb, in_=ins["baR"])
        waET_sb = []
        for k in range(KE):
            t_w = const.tile([128, F], BF16, name=f"waET{k}", tag=f"waET{k}")
            nc.sync.dma_start(out=t_w, in_=ins["WaET"][k * 128:(k + 1) * 128, :])
            waET_sb.append(t_w)
        w2_sb = const.tile([128, FJ], BF16, name="w2_sb", tag="w2")
        nc.sync.dma_start(out=w2_sb, in_=ins["w2T"])

        # encoder states, transposed: [e, b*T + t]; per-(k, b) tiles so the
        # first batch column can start computing before the full load lands
        enc_sb = [[None] * BL for _ in range(KE)]
        for b in range(BL):
            for k in range(KE):
                t_e = const.tile([128, T], BF16, name=f"enc{k}_{b}", tag=f"enc{k}_{b}")
                nc.sync.dma_start(
                    out=t_e,
                    in_=ins["encT"][k * 128:(k + 1) * 128, b * T:(b + 1) * T],
                )
                enc_sb[k][b] = t_e

        decT_sb = []
        for k in range(KE):
            t_d = const.tile([128, BL], BF16, name=f"decT{k}", tag=f"decT{k}")
            nc.sync.dma_start(out=t_d, in_=ins["decT"][k * 128:(k + 1) * 128, :])
            decT_sb.append(t_d)
        wcT_sb = []
        for k in range(KC):
            t_w = const.tile([128, D], BF16, name=f"wcT{k}", tag=f"wcT{k}")
            nc.sync.dma_start(out=t_w, in_=ins["WcT"][k * 128:(k + 1) * 128, :])
            wcT_sb.append(t_w)
        bc_sb = const.tile([1, D], BF16, name="bc_sb", tag="bc")
        nc.sync.dma_start(out=bc_sb, in_=ins["bcR"])

        # ---- h_proj = hidden @ WaD.T + ba  → [BL, F] then transpose ----
        h_proj = const.tile([BL, F], F32, name="h_proj", tag="h_proj")
        for c in range(F // 512):
            ph = misc_psum.tile([BL, 512], F32, name=f"ph{c}", tag="misc")
            for k in range(KE):
                nc.tensor.matmul(
                    ph, hT_sb[k], waDT_sb[k][:, c * 512:(c + 1) * 512],
                    start=(k == 0), stop=False,
                )
            nc.tensor.matmul(
                ph, ones, ba_sb[:, c * 512:(c + 1) * 512], start=False, stop=True,
            )
            nc.scalar.copy(h_proj[:, c * 512:(c + 1) * 512], ph)

        # h_projT[f, j, b] per-partition bias layout: [128, FJ, BL]
        h_projT = const.tile([128, FJ, BL], F32, name="h_projT", tag="h_projT")
        for j in range(FJ):
            pt = misc_psum.tile([128, BL], F32, name=f"pt{j}", tag="misc")
            nc.tensor.transpose(pt, h_proj[:, j * 128:(j + 1) * 128], ident[0:BL, 0:BL])
            nc.scalar.copy(h_projT[:, j, :], pt)

        # appliedT[e_tile][128, BL] accumulators (fp32)
        appT = []
        for k in range(KE):
            t_a = const.tile([128, BL], F32, name=f"appT{k}", tag=f"appT{k}")
            appT.append(t_a)

        # ---- main loop: per batch row, per f-tile ----
        for b in range(BL):
            ps = ps_psum.tile([1, T], F32, name=f"ps{b}", tag="ps")
            score_mm = []  # deferred score matmuls, emitted one j behind
            for j in range(FJ):
                pe = pe_psum.tile([128, T], F32, name=f"pe{b}_{j}", tag="pe")
                for k in range(KE):
                    nc.tensor.matmul(
                        pe,
                        waET_sb[k][:, j * 128:(j + 1) * 128],
                        enc_sb[k][b],
                        start=(k == 0), stop=(k == KE - 1),
                    )
                a_t = act_pool.tile([128, T], BF16, name=f"act{b}_{j}", tag="act")
                nc.scalar.activation(
                    a_t, pe, AF.Tanh, bias=h_projT[:, j, b:b + 1],
                )
                score_mm.append(a_t)
                # emit the score matmul for the *previous* j so PE doesn't
                # stall waiting on this j's tanh
                if j > 0:
                    nc.tensor.matmul(
                        ps, w2_sb[:, j - 1:j], score_mm[j - 1],
                        start=(j - 1 == 0), stop=False,
                    )
            nc.tensor.matmul(
                ps, w2_sb[:, FJ - 1:FJ], score_mm[FJ - 1], start=False, stop=True,
            )

            # softmax over t on a single partition
            negmax = sm_pool.tile([1, 1], F32, name=f"negmax{b}", tag="negmax")
            nc.vector.reduce_max(negmax, ps, axis=mybir.AxisListType.X, negate=True)
            wexp = sm_pool.tile([1, T], F32, name=f"wexp{b}", tag="wexp")
            sume = sm_pool.tile([1, 1], F32, name=f"sume{b}", tag="sume")
            nc.scalar.activation(wexp, ps, AF.Exp, bias=negmax, accum_out=sume)
            rsum = sm_pool.tile([1, 1], F32, name=f"rsum{b}", tag="rsum")
            nc.vector.reciprocal(rsum, sume)
            wnorm = sm_pool.tile([1, T], BF16, name=f"wnorm{b}", tag="wnorm")
            nc.vector.tensor_scalar_mul(wnorm, wexp, rsum)

            # broadcast weights to 128 partitions via DRAM round-trip
            nc.sync.dma_start(out=wscr[b:b + 1, :], in_=wnorm)
            wrep = wrep_pool.tile([128, T], BF16, name=f"wrep{b}", tag="wrep")
            row = wscr[b:b + 1, :]
            bsrc = bass.AP(tensor=row.tensor, offset=row.offset,
                           ap=[[0, 128]] + [list(p) for p in row.ap[1:]])
            nc.sync.dma_start(out=wrep, in_=bsrc)

            # appliedT[:, b] = sum_t enc * w
            for k in range(KE):
                scr = scr_pool.tile([128, T], BF16, name=f"scr{b}_{k}", tag="scr")
                nc.vector.scalar_tensor_tensor(
                    out=scr, in0=enc_sb[k][b], scalar=1.0, in1=wrep,
                    op0=ALU.mult, op1=ALU.mult,
                    accum_out=appT[k][:, b:b + 1],
                )

        # ---- epilogue: applied output + combine matmul ----
        applied_sb = const.tile([BL, E], F32, name="applied_sb", tag="applied_sb")
        appT_bf = []
        for k in range(KE):
            t_c = const.tile([128, BL], BF16, name=f"appBf{k}", tag=f"appBf{k}")
            nc.vector.tensor_copy(t_c, appT[k])
            appT_bf.append(t_c)
            pa = misc_psum.tile([BL, 128], F32, name=f"pa{k}", tag="misc")
            nc.tensor.transpose(pa, appT[k], ident)
            nc.scalar.copy(applied_sb[:, k * 128:(k + 1) * 128], pa)
        nc.sync.dma_start(out=app_d, in_=applied_sb)

        out_sb = const.tile([BL, D], F32, name="out_sb", tag="out_sb")
        for h in range(D // 512):
            pc = misc_psum.tile([BL, 512], F32, name=f"pc{h}", tag="misc")
            for k in range(KC):
                lhs = decT_sb[k] if k < KE else appT_bf[k - KE]
                nc.tensor.matmul(
                    pc, lhs, wcT_sb[k][:, h * 512:(h + 1) * 512],
                    start=(k == 0), stop=False,
                )
            nc.tensor.matmul(
                pc, ones, bc_sb[:, h * 512:(h + 1) * 512], start=False, stop=True,
            )
            nc.scalar.activation(out_sb[:, h * 512:(h + 1) * 512], pc, AF.Tanh)
        nc.sync.dma_start(out=out_d, in_=out_sb)


def build_nc(reps=1):
    nc = bacc.Bacc("TRN2", target_bir_lowering=False, debug=False)
    ins = {}

    def din(name, shape, dt=BF16):
        ins[name] = nc.dram_tensor(name, shape, dt, kind="ExternalInput").ap()

    din("encT", [E, BL * T])
    din("hT", [D, BL])
    din("decT", [D, BL])
    din("WaDT", [D, F])
    din("WaET", [E, F])
    din("WcT", [D + E, D])
    din("w2T", [128, FJ])
    din("baR", [1, F])
    din("bcR", [1, D])
    wscr = nc.dram_tensor("wscr", [BL, T], BF16, kind="Internal").ap()
    out_d = nc.dram_tensor("out", [BL, D], F32, kind="ExternalOutput").ap()
    app_d = nc.dram_tensor("applied", [BL, E], F32, kind="ExternalOutput").ap()
    with tile.TileContext(nc) as tc:
        for _ in range(reps):
            _body(tc, ins, wscr, out_d, app_d)
    nc.compile()
    return nc


def _prep_inputs(hidden, decoder_out, encoder_states, Wa, ba, w2, Wc, bc):
    bf = ml_dtypes.bfloat16
    f32 = np.float32

    def to_bf(a):
        return np.ascontiguousarray(np.asarray(a, f32)).astype(bf)

    shared = {
        "WaDT": np.ascontiguousarray(np.asarray(Wa[:, :D], f32).T).astype(bf),
        "WaET": np.ascontiguousarray(np.asarray(Wa[:, D:], f32).T).astype(bf),
        "WcT": np.ascontiguousarray(np.asarray(Wc, f32).T).astype(bf),
        "w2T": np.ascontiguousarray(
            np.asarray(w2[0], f32).reshape(FJ, 128).T).astype(bf),
        "baR": to_bf(np.asarray(ba, f32).reshape(1, F)),
        "bcR": to_bf(np.asarray(bc, f32).reshape(1, D)),
    }
    enc_bf = np.asarray(encoder_states, f32).astype(bf)  # [T, B, E]
    in_maps = []
    for c in range(NCORES):
        sl = slice(c * BL, (c + 1) * BL)
        encT = np.ascontiguousarray(
            enc_bf[:, sl, :].transpose(2, 1, 0)).reshape(E, BL * T)
        m = dict(shared)
        m["encT"] = encT
        m["hT"] = np.ascontiguousarray(np.asarray(hidden[sl], f32).T).astype(bf)
        m["decT"] = np.ascontiguousarray(np.asarray(decoder_out[sl], f32).T).astype(bf)
        in_maps.append(m)
    return in_maps


def kernel(hidden, decoder_out, encoder_states, Wa, ba, w2, b2, Wc, bc):
    global _nc_cache
    if _nc_cache is None:
        _nc_cache = build_nc()
    in_maps = _prep_inputs(hidden, decoder_out, encoder_states, Wa, ba, w2, Wc, bc)
    res = run_bass_kernel_spmd(_nc_cache, in_maps, core_ids=list(range(NCORES)))
    out = np.concatenate([res.results[c]["out"] for c in range(NCORES)], axis=0)
    applied = np.concatenate(
        [res.results[c]["applied"] for c in range(NCORES)], axis=0)
    return out.astype(np.float32), applied.astype(np.float32)


# revision 9
# speedup vs baseline: 171.6388x; 1.4169x over previous
"""Trainium2 Bass/Tile kernel for the AttentionModule problem.

Computation (per the reference):
    h_proj  = hidden @ Wa[:, :D].T + ba                       [B, 2E]
    e_proj  = einsum('tbe,fe->tbf', enc, Wa[:, D:])           [T, B, 2E]
    act     = tanh(h_proj + e_proj)
    scores  = einsum('tbf,f->bt', act, w2[0]) (+ b2, dropped — softmax invariant)
    weights = softmax(scores, axis=t)
    applied = einsum('bt,tbe->be', weights, enc)              [B, E]
    out     = tanh(cat(decoder_out, applied) @ Wc.T + bc)     [B, D]

Strategy: data-parallel over batch B=64 across 8 NeuronCores (8 rows each).
All matmul operands are host-pre-transposed so the contraction dim (e/d/k)
lands on SBUF partitions; inputs are cast to bf16 on host (fp32 PSUM accum).

Per-core device layout (f = 2E on PSUM partitions for the big matmul):
    pre[f_tile=128, t=512]  = sum_e WaET[e, f].T @ encT[e, (b,t)]  (8 K-tiles)
    act = tanh(pre + h_projT[f, b])        (ACT, bias = per-partition scalar)
    scores[1, t] += w2[f_tile].T @ act     (PE, M=1, accumulated over 16 f-tiles)
    softmax on one partition, weights broadcast to 128 partitions via DMA
    appliedT[e_tile, b] = reduce_t(encT * wrep)   (DVE scalar_tensor_tensor)
    out[b, :] = tanh(cat(decT, appliedT).T @ WcT + bc)  (PE + ACT)
"""

import numpy as np
import ml_dtypes
from contextlib import ExitStack

import concourse.bass as bass
import concourse.tile as tile
from concourse import bacc, mybir
from concourse.bass_utils import run_bass_kernel_spmd
from concourse.masks import make_identity

B, T, E, D = 64, 512, 1024, 1024
NCORES = 8
BL = B // NCORES          # 8 batch rows per core
F = 2 * E                 # 2048
KE = E // 128             # 8 contraction tiles for e/d
KC = (D + E) // 128       # 16 contraction tiles for the combine matmul
FJ = F // 128             # 16 f-tiles
BF16 = mybir.dt.bfloat16
F32 = mybir.dt.float32
AF = mybir.ActivationFunctionType
ALU = mybir.AluOpType

_nc_cache = None


def _load_consts(tc, ctx, ins, uid=""):
    """Load all weights + encoder states into SBUF. Returns tile dict."""
    nc = tc.nc
    const = ctx.enter_context(tc.tile_pool(name=f"const{uid}", bufs=1))
    tl = {}
    tl["ones"] = const.tile([1, BL], BF16, name="ones", tag="ones")
    nc.vector.memset(tl["ones"], 1.0)
    tl["ident"] = const.tile([128, 128], F32, name="ident", tag="ident")
    make_identity(nc, tl["ident"])

    tl["hT"] = []
    tl["waDT"] = []
    for k in range(KE):
        t_h = const.tile([128, BL], BF16, name=f"hT{k}", tag=f"hT{k}")
        nc.sync.dma_start(out=t_h, in_=ins["hT"][k * 128:(k + 1) * 128, :])
        tl["hT"].append(t_h)
        t_w = const.tile([128, F], BF16, name=f"waDT{k}", tag=f"waDT{k}")
        nc.sync.dma_start(out=t_w, in_=ins["WaDT"][k * 128:(k + 1) * 128, :])
        tl["waDT"].append(t_w)
    tl["ba"] = const.tile([1, F], BF16, name="ba_sb", tag="ba")
    nc.sync.dma_start(out=tl["ba"], in_=ins["baR"])
    tl["waET"] = []
    for k in range(KE):
        t_w = const.tile([128, F], BF16, name=f"waET{k}", tag=f"waET{k}")
        nc.sync.dma_start(out=t_w, in_=ins["WaET"][k * 128:(k + 1) * 128, :])
        tl["waET"].append(t_w)
    tl["w2"] = const.tile([128, FJ], BF16, name="w2_sb", tag="w2")
    nc.sync.dma_start(out=tl["w2"], in_=ins["w2T"])

    # encoder states, transposed: [b, e, t]; per-(k, b) tiles are contiguous
    # 128 KiB blocks, and the first batch column can start computing before
    # the full load lands
    tl["enc"] = [[None] * BL for _ in range(KE)]
    for b in range(BL):
        for k in range(KE):
            t_e = const.tile([128, T], BF16, name=f"enc{k}_{b}", tag=f"enc{k}_{b}")
            nc.sync.dma_start(
                out=t_e,
                in_=ins["encT"][b, k * 128:(k + 1) * 128, :],
            )
            tl["enc"][k][b] = t_e

    tl["decT"] = []
    for k in range(KE):
        t_d = const.tile([128, BL], BF16, name=f"decT{k}", tag=f"decT{k}")
        nc.sync.dma_start(out=t_d, in_=ins["decT"][k * 128:(k + 1) * 128, :])
        tl["decT"].append(t_d)
    tl["wcT"] = []
    for k in range(KC):
        t_w = const.tile([128, D], BF16, name=f"wcT{k}", tag=f"wcT{k}")
        nc.sync.dma_start(out=t_w, in_=ins["WcT"][k * 128:(k + 1) * 128, :])
        tl["wcT"].append(t_w)
    tl["bc"] = const.tile([1, D], BF16, name="bc_sb", tag="bc")
    nc.sync.dma_start(out=tl["bc"], in_=ins["bcR"])
    return tl


def _compute(tc, ctx, tl, wscr, out_d, app_d, uid=""):
    nc = tc.nc
    work = ctx.enter_context(tc.tile_pool(name=f"work{uid}", bufs=1))
    act_pool = ctx.enter_context(tc.tile_pool(name=f"actp{uid}", bufs=3))
    wrep_pool = ctx.enter_context(tc.tile_pool(name=f"wrepp{uid}", bufs=2))
    scr_pool = ctx.enter_context(tc.tile_pool(name=f"scrp{uid}", bufs=2))
    sm_pool = ctx.enter_context(tc.tile_pool(name=f"smp{uid}", bufs=2))
    pe_psum = ctx.enter_context(tc.tile_pool(name=f"pep{uid}", bufs=3, space="PSUM"))
    ps_psum = ctx.enter_context(tc.tile_pool(name=f"psp{uid}", bufs=2, space="PSUM"))
    misc_psum = ctx.enter_context(
        tc.tile_pool(name=f"mip{uid}", bufs=2, space="PSUM"))

    ones, ident = tl["ones"], tl["ident"]

    # ---- h_proj = hidden @ WaD.T + ba  → [BL, F] then transpose ----
    h_proj = work.tile([BL, F], F32, name="h_proj", tag="h_proj")
    for c in range(F // 512):
        ph = misc_psum.tile([BL, 512], F32, name=f"ph{c}", tag="misc")
        for k in range(KE):
            nc.tensor.matmul(
                ph, tl["hT"][k], tl["waDT"][k][:, c * 512:(c + 1) * 512],
                start=(k == 0), stop=False,
            )
        nc.tensor.matmul(
            ph, ones, tl["ba"][:, c * 512:(c + 1) * 512], start=False, stop=True,
        )
        nc.scalar.copy(h_proj[:, c * 512:(c + 1) * 512], ph)

    # h_projT[f, j, b] per-partition bias layout: [128, FJ, BL]
    h_projT = work.tile([128, FJ, BL], F32, name="h_projT", tag="h_projT")
    for j in range(FJ):
        pt = misc_psum.tile([128, BL], F32, name=f"pt{j}", tag="misc")
        nc.tensor.transpose(pt, h_proj[:, j * 128:(j + 1) * 128], ident[0:BL, 0:BL])
        nc.scalar.copy(h_projT[:, j, :], pt)

    # appliedT[e_tile][128, BL] accumulators (fp32)
    appT = []
    for k in range(KE):
        appT.append(work.tile([128, BL], F32, name=f"appT{k}", tag=f"appT{k}"))

    # ---- main loop: per batch row, per f-tile ----
    for b in range(BL):
        ps = ps_psum.tile([1, T], F32, name=f"ps{b}", tag="ps")
        acts = []
        for j in range(FJ):
            pe = pe_psum.tile([128, T], F32, name=f"pe{b}_{j}", tag="pe")
            for k in range(KE):
                nc.tensor.matmul(
                    pe,
                    tl["waET"][k][:, j * 128:(j + 1) * 128],
                    tl["enc"][k][b],
                    start=(k == 0), stop=(k == KE - 1),
                )
            a_t = act_pool.tile([128, T], BF16, name=f"act{b}_{j}", tag="act")
            nc.scalar.activation(a_t, pe, AF.Tanh, bias=h_projT[:, j, b:b + 1])
            acts.append(a_t)
            # emit the score matmul one j behind so PE doesn't stall on tanh
            if j > 0:
                nc.tensor.matmul(ps, tl["w2"][:, j - 1:j], acts[j - 1],
                                 start=(j - 1 == 0), stop=False)
        nc.tensor.matmul(ps, tl["w2"][:, FJ - 1:FJ], acts[FJ - 1],
                         start=False, stop=True)

        # softmax over t on a single partition
        negmax = sm_pool.tile([1, 1], F32, name=f"negmax{b}", tag="negmax")
        nc.vector.reduce_max(negmax, ps, axis=mybir.AxisListType.X, negate=True)
        wexp = sm_pool.tile([1, T], F32, name=f"wexp{b}", tag="wexp")
        sume = sm_pool.tile([1, 1], F32, name=f"sume{b}", tag="sume")
        nc.scalar.activation(wexp, ps, AF.Exp, bias=negmax, accum_out=sume)
        rsum = sm_pool.tile([1, 1], F32, name=f"rsum{b}", tag="rsum")
        nc.vector.reciprocal(rsum, sume)
        wnorm = sm_pool.tile([1, T], BF16, name=f"wnorm{b}", tag="wnorm")
        nc.vector.tensor_scalar_mul(wnorm, wexp, rsum)

        # broadcast weights to 128 partitions via DRAM round-trip
        nc.sync.dma_start(out=wscr[b:b + 1, :], in_=wnorm)
        wrep = wrep_pool.tile([128, T], BF16, name=f"wrep{b}", tag="wrep")
        row = wscr[b:b + 1, :]
        bsrc = bass.AP(tensor=row.tensor, offset=row.offset,
                       ap=[[0, 128]] + [list(p) for p in row.ap[1:]])
        nc.sync.dma_start(out=wrep, in_=bsrc)

        # appliedT[:, b] = sum_t enc * w
        for k in range(KE):
            scr = scr_pool.tile([128, T], BF16, name=f"scr{b}_{k}", tag="scr")
            nc.vector.scalar_tensor_tensor(
                out=scr, in0=tl["enc"][k][b], scalar=1.0, in1=wrep,
                op0=ALU.mult, op1=ALU.mult,
                accum_out=appT[k][:, b:b + 1],
            )

    # ---- epilogue: applied output + combine matmul ----
    applied_sb = work.tile([BL, E], F32, name="applied_sb", tag="applied_sb")
    appT_bf = []
    for k in range(KE):
        t_c = work.tile([128, BL], BF16, name=f"appBf{k}", tag=f"appBf{k}")
        nc.vector.tensor_copy(t_c, appT[k])
        appT_bf.append(t_c)
        pa = misc_psum.tile([BL, 128], F32, name=f"pa{k}", tag="misc")
        nc.tensor.transpose(pa, appT[k], ident)
        nc.scalar.copy(applied_sb[:, k * 128:(k + 1) * 128], pa)
    nc.sync.dma_start(out=app_d, in_=applied_sb)

    out_sb = work.tile([BL, D], F32, name="out_sb", tag="out_sb")
    for h in range(D // 512):
        pc = misc_psum.tile([BL, 512], F32, name=f"pc{h}", tag="misc")
        for k in range(KC):
            lhs = tl["decT"][k] if k < KE else appT_bf[k - KE]
            nc.tensor.matmul(
                pc, lhs, tl["wcT"][k][:, h * 512:(h + 1) * 512],
                start=(k == 0), stop=False,
            )
        nc.tensor.matmul(
            pc, ones, tl["bc"][:, h * 512:(h + 1) * 512], start=False, stop=True,
        )
        nc.scalar.activation(out_sb[:, h * 512:(h + 1) * 512], pc, AF.Tanh)
    nc.sync.dma_start(out=out_d, in_=out_sb)


def build_nc(reps=1, mode="full"):
    """mode: 'full' = load+compute per rep; 'compute1' = load once, compute
    `reps` times; 'dma' = load only, `reps` times."""
    nc = bacc.Bacc("TRN2", target_bir_lowering=False, debug=False)
    ins = {}

    def din(name, shape, dt=BF16):
        ins[name] = nc.dram_tensor(name, shape, dt, kind="ExternalInput").ap()

    din("encT", [BL, E, T])
    din("hT", [D, BL])
    din("decT", [D, BL])
    din("WaDT", [D, F])
    din("WaET", [E, F])
    din("WcT", [D + E, D])
    din("w2T", [128, FJ])
    din("baR", [1, F])
    din("bcR", [1, D])
    wscr = nc.dram_tensor("wscr", [BL, T], BF16, kind="Internal").ap()
    out_d = nc.dram_tensor("out", [BL, D], F32, kind="ExternalOutput").ap()
    app_d = nc.dram_tensor("applied", [BL, E], F32, kind="ExternalOutput").ap()
    with tile.TileContext(nc) as tc:
        if mode == "full":
            for r in range(reps):
                with ExitStack() as ctx:
                    tl = _load_consts(tc, ctx, ins, uid=f"r{r}")
                    _compute(tc, ctx, tl, wscr, out_d, app_d, uid=f"r{r}")
        elif mode == "compute1":
            with ExitStack() as octx:
                tl = _load_consts(tc, octx, ins)
                for r in range(reps):
                    with ExitStack() as ctx:
                        _compute(tc, ctx, tl, wscr, out_d, app_d, uid=f"r{r}")
        elif mode == "dma":
            for r in range(reps):
                with ExitStack() as ctx:
                    tl = _load_consts(tc, ctx, ins, uid=f"r{r}")
                    # touch one tile so loads aren't dead-code eliminated
                    s = ctx.enter_context(tc.tile_pool(name=f"s{r}", bufs=1))
                    acc = s.tile([128, 1], F32, name="acc", tag="acc")
                    touch = ([t for row in tl["enc"] for t in row]
                             + tl["waDT"] + tl["waET"] + tl["wcT"] + tl["hT"]
                             + tl["decT"])
                    for i, t in enumerate(touch):
                        nc.vector.reduce_max(acc, t[:, 0:1],
                                             axis=mybir.AxisListType.X)
                    nc.sync.dma_start(out=out_d[0:1, 0:128],
                                      in_=acc.rearrange("p one -> one p"))
        else:
            raise ValueError(mode)
    nc.compile()
    return nc


def _prep_inputs(hidden, decoder_out, encoder_states, Wa, ba, w2, Wc, bc):
    bf = ml_dtypes.bfloat16
    f32 = np.float32

    def to_bf(a):
        return np.ascontiguousarray(np.asarray(a, f32)).astype(bf)

    shared = {
        "WaDT": np.ascontiguousarray(np.asarray(Wa[:, :D], f32).T).astype(bf),
        "WaET": np.ascontiguousarray(np.asarray(Wa[:, D:], f32).T).astype(bf),
        "WcT": np.ascontiguousarray(np.asarray(Wc, f32).T).astype(bf),
        "w2T": np.ascontiguousarray(
            np.asarray(w2[0], f32).reshape(FJ, 128).T).astype(bf),
        "baR": to_bf(np.asarray(ba, f32).reshape(1, F)),
        "bcR": to_bf(np.asarray(bc, f32).reshape(1, D)),
    }
    enc_bf = np.asarray(encoder_states, f32).astype(bf)  # [T, B, E]
    in_maps = []
    for c in range(NCORES):
        sl = slice(c * BL, (c + 1) * BL)
        encT = np.ascontiguousarray(enc_bf[:, sl, :].transpose(1, 2, 0))
        m = dict(shared)
        m["encT"] = encT
        m["hT"] = np.ascontiguousarray(np.asarray(hidden[sl], f32).T).astype(bf)
        m["decT"] = np.ascontiguousarray(np.asarray(decoder_out[sl], f32).T).astype(bf)
        in_maps.append(m)
    return in_maps


def kernel(hidden, decoder_out, encoder_states, Wa, ba, w2, b2, Wc, bc):
    global _nc_cache
    if _nc_cache is None:
        _nc_cache = build_nc()
    in_maps = _prep_inputs(hidden, decoder_out, encoder_states, Wa, ba, w2, Wc, bc)
    res = run_bass_kernel_spmd(_nc_cache, in_maps, core_ids=list(range(NCORES)))
    out = np.concatenate([res.results[c]["out"] for c in range(NCORES)], axis=0)
    applied = np.concatenate(
        [res.results[c]["applied"] for c in range(NCORES)], axis=0)
    return out.astype(np.float32), applied.astype(np.float32)
